# revision 29
# baseline (speedup 1.0000x reference)
"""Trainium2 Bass kernel for nn_FFTPermeabilityPredictorPatchPhysics.

Sharding: pure data parallel — 8 samples per NeuronCore, weights replicated.
On-device layout: residual stream transposed, hT [3x128 d-chunks, 1576 tok],
kept in SBUF for all 12 layers. FFT/iFFT as block-diagonal matmuls over a
512-row padded frequency layout (head h -> rows 64h+32s+f). Matmuls run
float32r (full PE rate, ~11-bit mantissa) except the MLP which runs bf16.
LN stats via ones-matmul partition reductions broadcast to all partitions;
the adaptive spectral filter is fused into the ACT-engine gelu via
per-partition scale/bias. All weight folding done host-side in numpy:
double-LN collapse, pre_g/ln2_g into following matmuls, base_filter and
(1+ap) into amlp_w2, 1/197 token-mean into amlp_w1, DFT matrices baked.
Final LN + head on the 64 cls vectors runs host-side in float64.
"""
import numpy as np

import concourse.bacc as bacc
import concourse.mybir as mybir
import concourse.tile as tile
from concourse.bass_utils import run_bass_kernel_spmd

F32 = mybir.dt.float32
F32R = mybir.dt.float32r
BF16 = mybir.dt.bfloat16
AF = mybir.ActivationFunctionType
ALU = mybir.AluOpType

B, D, H, HD, FB, S, L, P, NP_ = 64, 384, 8, 48, 25, 197, 12, 16, 196
EPS = 1e-5
FR = 512
NCORES = 8
BC = B // NCORES     # 8 samples/core
NTOK = BC * S        # 1576
TT = 394             # token tile = 2 samples
NBP = BC * NP_       # 1568
BT = 392             # patch tile = 2 samples

_CACHE = {}
BUFS_HR = 8
BUFS_ST = 4
BUFS_FG = 2
BUFS_MID = 2
BUFS_H2 = 4


def _build_dft():
    n = np.arange(HD)
    k = np.arange(FB)
    ang = -2 * np.pi * np.outer(n, k) / HD
    Cr = np.cos(ang) / np.sqrt(HD)
    Ci = np.sin(ang) / np.sqrt(HD)
    A = np.zeros((FB, HD))
    Bm = np.zeros((FB, HD))
    ifft_w = np.exp(2j * np.pi * np.outer(np.arange(HD), np.arange(HD)) / HD) / np.sqrt(HD)
    for j in range(FB):
        fr = np.zeros(HD, complex)
        fi = np.zeros(HD, complex)
        fr[j] = 1.0
        fi[j] = 1.0j
        if 0 < j < HD - FB + 1:
            fr[HD - j] = 1.0
            fi[HD - j] = -1.0j
        A[j] = (ifft_w @ fr).real
        Bm[j] = (ifft_w @ fi).real
    return Cr, Ci, A, Bm


def _prep(inp, n_layers=L):
    f = {k: np.asarray(v, np.float64) for k, v in inp.items()}
    Cr, Ci, A, Bm = _build_dft()

    BDb = np.zeros((D, FR))
    iBD = np.zeros((FR, D))
    for h in range(H):
        BDb[48 * h:48 * h + 48, 64 * h:64 * h + FB] = Cr
        BDb[48 * h:48 * h + 48, 64 * h + 32:64 * h + 32 + FB] = Ci
        iBD[64 * h:64 * h + FB, 48 * h:48 * h + 48] = A
        iBD[64 * h + 32:64 * h + 32 + FB, 48 * h:48 * h + 48] = Bm

    cg = f['ln1_g'].mean(1)
    assert np.abs(f['ln1_g'] - cg[:, None]).max() < 1e-12, "ln1_g must be constant/layer"
    assert np.abs(f['ln1_b'] - f['ln1_b'].mean(1)[:, None]).max() < 1e-12
    assert np.allclose(f['pe_ln_g'], 1.0) and np.allclose(f['pe_ln_b'], 0.0), "pe_ln fold"

    BD_l = np.einsum('ld,df->ldf', cg[:, None] * f['pre_g'], BDb)
    bdbias_l = np.einsum('ld,df->lf', f['pre_b'], BDb)

    aw1p = np.einsum('ld,lde->lde', cg[:, None] * f['pre_g'], f['amlp_w1']) / S
    ab1p = np.einsum('ld,lde->le', f['pre_b'], f['amlp_w1']) + f['amlp_b1']

    aw2pp = np.zeros((L, D, 2 * FR))
    ab2pp = np.zeros((L, 2 * FR))
    aw2, ab2 = f['amlp_w2'], f['amlp_b2']
    bf, bb = f['base_filter'], f['base_bias']
    for h in range(H):
        for s in range(2):
            for fq in range(FB):
                r = 64 * h + 32 * s + fq
                c0 = h * (FB * 2) + fq * 2
                wf = bf[:, h, fq][:, None] * aw2[:, :, c0]
                bf_ = bf[:, h, fq] * ab2[:, c0] + bf[:, h, fq]
                aw2pp[:, :, r] = wf
                ab2pp[:, r] = bf_
                aw2pp[:, :, FR + r] = bdbias_l[:, r][:, None] * wf
                ab2pp[:, FR + r] = bdbias_l[:, r] * bf_
                if s == 0:
                    aw2pp[:, :, FR + r] += aw2[:, :, c0 + 1]
                    ab2pp[:, FR + r] += bb[:, h, fq] + ab2[:, c0 + 1]

    w1p = np.einsum('ld,lde->lde', f['ln2_g'], f['mlp_w1'])
    b1p = np.einsum('ld,lde->le', f['ln2_b'], f['mlp_w1']) + f['mlp_b1']

    a32 = lambda x: np.ascontiguousarray(x, np.float32)
    g = {}
    g['cg'] = cg
    g['W1'] = _bf16(w1p.reshape(L, 3, 128, 4 * D).transpose(0, 2, 1, 3))            # [L,128,3,1536] bf16
    g['W2'] = _bf16(f['mlp_w2'].reshape(L, 12, 128, 3, 128).transpose(0, 2, 1, 3, 4))
    g['BD'] = a32(BD_l.reshape(L, 3, 128, 4, 128).transpose(0, 2, 1, 3, 4))
    g['IBD'] = a32(iBD.reshape(4, 128, 3, 128).transpose(1, 0, 2, 3))
    g['AW1'] = a32(aw1p.reshape(L, 3, 128, D).transpose(0, 2, 1, 3))
    g['AB2R'] = a32(ab2pp[:, None, :])                                            # [L,1,1024]
    g['B2R'] = _bf16(f['mlp_b2'][:, None, :].reshape(L, 1, 3, 128))
    g['AW2'] = a32(aw2pp.reshape(L, 3, 128, 2 * FR).transpose(0, 2, 1, 3))
    # packed per-layer biases [L,128,26]: 0-2 ab1, 3-10 ab2, 11-22 b1, 23-25 b2
    bias = np.zeros((L, 128, 26))
    bias[:, :, 0:3] = ab1p.reshape(L, 3, 128).transpose(0, 2, 1)
    bias[:, :, 3:11] = ab2pp.reshape(L, 8, 128).transpose(0, 2, 1)
    bias[:, :, 11:23] = b1p.reshape(L, 12, 128).transpose(0, 2, 1)
    bias[:, :, 23:26] = f['mlp_b2'].reshape(L, 3, 128).transpose(0, 2, 1)
    g['BIAS'] = a32(bias)
    g['PEW'] = a32(f['pe_w'].reshape(3, 2, 128, 128).transpose(2, 0, 1, 3))          # [128,3,2,128]
    g['PHW'] = a32(f['phys_w'].reshape(6, 3, 128))                                   # [6,3,128]
    g['GW'] = a32(f['gate_w'].reshape(6, 128, 3, 128).transpose(1, 0, 2, 3))         # [128,6,3,128]
    fbias = np.zeros((128, 12))  # 0-2 peb, 3-5 phb, 6-8 gb, 9-11 clspe
    fbias[:, 0:3] = f['pe_b'].T
    fbias[:, 3:6] = f['phys_b'].reshape(3, 128).T
    fbias[:, 6:9] = f['gate_b'].reshape(3, 128).T
    fbias[:, 9:12] = (f['cls_token'][0, 0] + f['pos_embed'][0, 0]).reshape(3, 128).T
    g['FBIAS'] = a32(fbias)
    g['PET'] = a32(f['pos_embed'][0, 1:].T.reshape(3, 128, NP_).transpose(1, 0, 2))  # [128,3,196]
    for kk in ('norm_g', 'norm_b', 'head_w1', 'head_b1', 'head_w2', 'head_b2'):
        g[kk] = f[kk]
    g['n_layers'] = n_layers
    return g


def _bf16(x):
    import ml_dtypes
    return np.ascontiguousarray(np.asarray(x, np.float32), dtype=ml_dtypes.bfloat16)


def _build(g):
    n_layers = g['n_layers']
    nc = bacc.Bacc('TRN2', target_bir_lowering=False, debug=False)
    # register float constants used as ACT biases
    for val in (EPS,):
        t = nc.alloc_sbuf_tensor(f"const-f32-{val}", [128, 1], F32)
        nc.gpsimd.memset(t.ap(), val)
        nc.const_aps.aps[(F32, val)] = t.ap()
    nc.all_engine_barrier()

    di = lambda name, shape, dt: nc.dram_tensor(name, list(shape), dt, kind="ExternalInput")
    PATd = di('patt', (128, 3, 2, NBP), F32R)
    PFT = di('pft', (6, NBP), F32R)
    W1d = di('w1', (L, 128, 3, 1536), BF16)
    W2d = di('w2', (L, 128, 12, 3, 128), BF16)
    BDd = di('bd', (L, 128, 3, 4, 128), F32R)
    IBDd = di('ibd', (128, 4, 3, 128), F32R)
    AW1d = di('aw1', (L, 128, 3, 384), F32)
    AW2d = di('aw2', (L, 128, 3, 1024), F32)
    BIASd = di('bias', (L, 128, 26), F32)
    AB2Rd = di('ab2r', (L, 1, 1024), F32)
    B2Rd = di('b2r', (L, 1, 3, 128), BF16)
    ONFd = di('onesf', (1, BC), F32)
    ONBd = di('onesb', (1, TT), BF16)
    PEWd = di('pew', (128, 3, 2, 128), F32R)
    PHWd = di('phw', (6, 3, 128), F32R)
    GWd = di('gw', (128, 6, 3, 128), F32R)
    FBIASd = di('fbias', (128, 12), F32)
    PETd = di('pet', (128, 3, NP_), F32)
    ONESd = di('ones', (128, 128), F32R)
    HCLS = nc.dram_tensor('hcls', [128, 3, BC], F32, kind="ExternalOutput")

    with tile.TileContext(nc) as tc:
        with (
            tc.tile_pool(name='const', bufs=1) as cp,
            tc.tile_pool(name='persist', bufs=1) as pp,
            tc.tile_pool(name='hnp', bufs=1) as hnp,
            tc.tile_pool(name='hrp', bufs=BUFS_HR) as hrp,
            tc.tile_pool(name='stp', bufs=BUFS_ST) as stp,
            tc.tile_pool(name='psp', bufs=8, space='PSUM') as psp,
        ):
            ones_t = cp.tile([128, 128], F32R, name='ones_t')
            nc.sync.dma_start(ones_t[:], ONESd[:])
            ibd_t = cp.tile([128, 4, 3, 128], F32R, name='ibd_t')
            nc.sync.dma_start(ibd_t[:], IBDd[:])
            onesf_t = cp.tile([1, BC], F32, name='onesf_t')
            nc.sync.dma_start(onesf_t[:], ONFd[:])
            onesb_t = cp.tile([1, TT], BF16, name='onesb_t')
            nc.sync.dma_start(onesb_t[:], ONBd[:])
            fbias_t = cp.tile([128, 12], F32, name='fbias_t')
            nc.sync.dma_start(fbias_t[:], FBIASd[:])
            pet_t = cp.tile([128, 3, NP_], F32, name='pet_t')
            nc.sync.dma_start(pet_t[:], PETd[:])

            hT = pp.tile([128, 3, NTOK], F32, name='hT')

            def ln_stats(srcs, tlen, cgl=None, pstag='ps'):
                """LN stats for one token tile; srcs = 3 [128,tlen] f32 APs.
                Double-LN folds to a single rsqrt:
                rs1*rs2 = rsqrt((cg^2+eps)*v + eps^2). Returns (st, m, rsd)."""
                hrs = []
                for c in range(3):
                    hr = hrp.tile([128, TT], F32R, tag='hr', name='hr')
                    xq = hrp.tile([128, TT], F32R, tag='xq', name='xq')
                    nc.gpsimd.tensor_scalar(hr[:, :tlen], srcs[c], 1.0, None, ALU.mult)
                    eng_q = nc.gpsimd if c == 2 else nc.vector
                    eng_q.tensor_mul(xq[:, :tlen], srcs[c], srcs[c])
                    hrs.append((hr, xq))
                ps_s = psp.tile([128, TT], F32, tag=pstag, name='ps_s')
                ps_q = psp.tile([128, TT], F32, tag=pstag, name='ps_q')
                for c in range(3):
                    nc.tensor.matmul(ps_s[:, :tlen], ones_t[:], hrs[c][0][:, :tlen],
                                     start=(c == 0), stop=(c == 2))
                for c in range(3):
                    nc.tensor.matmul(ps_q[:, :tlen], ones_t[:], hrs[c][1][:, :tlen],
                                     start=(c == 0), stop=(c == 2))
                if cgl is None:
                    A, Bc_ = 1.0, EPS
                else:
                    A = float(cgl) * float(cgl) + EPS
                    Bc_ = EPS * EPS
                import math
                sA = math.sqrt(A)
                st = stp.tile([128, 6, TT], F32, tag='st', name='st')
                m = st[:, 0, :tlen]
                mA = st[:, 1, :tlen]
                msqA = st[:, 2, :tlen]
                t1 = st[:, 3, :tlen]
                ve = st[:, 4, :tlen]
                u = st[:, 5, :tlen]
                rsd = st[:, 1, :tlen]   # mA dead after msqA
                nc.vector.tensor_scalar(m, ps_s[:, :tlen], 1.0 / D, None, ALU.mult)
                nc.vector.tensor_scalar(mA, ps_s[:, :tlen], sA / D, None, ALU.mult)
                nc.vector.tensor_mul(msqA, mA, mA)
                nc.vector.tensor_scalar(t1, ps_q[:, :tlen], A / D, Bc_, ALU.mult, ALU.add)
                nc.vector.tensor_sub(ve, t1, msqA)
                nc.vector.reciprocal(u, ve)
                nc.scalar.activation(rsd, u, AF.Sqrt)
                return st, m, rsd

            def ln_apply(st, m, rsd, srcs, dsts, tlen):
                tmp = st[:, 2, :tlen]
                tmp2 = st[:, 3, :tlen]
                for c in range(3):
                    eng = nc.gpsimd if c == 2 else nc.vector
                    tm = tmp2 if c == 2 else tmp
                    eng.tensor_sub(tm, srcs[c], m)
                    eng.tensor_mul(dsts[c], tm, rsd)

            # ================= front (streamed per 2-sample group) ==========
            with (
                tc.tile_pool(name='fgrp', bufs=2) as fg_,
                tc.tile_pool(name='fw', bufs=1) as fw,
            ):
                pft_t = fw.tile([6, NBP], F32R, name='pft_t')
                nc.sync.dma_start(pft_t[:], PFT[:])
                pew_t = fw.tile([128, 3, 2, 128], F32R, name='pew_t')
                nc.sync.dma_start(pew_t[:], PEWd[:])
                phw_t = fw.tile([6, 3, 128], F32R, name='phw_t')
                nc.sync.dma_start(phw_t[:], PHWd[:])
                gw_t = fw.tile([128, 6, 3, 128], F32R, name='gw_t')
                nc.sync.dma_start(gw_t[:], GWd[:])

                for grp in range(4):
                    sl = slice(grp * BT, (grp + 1) * BT)
                    patg = fg_.tile([128, 3, 2, BT], F32R, tag='patg', name='patg')
                    nc.sync.dma_start(patg[:], PATd[:, :, :, sl])
                    ximg = fg_.tile([128, 3, BT], F32R, tag='ximg', name='ximg')
                    xn = fg_.tile([128, 3, BT], F32R, tag='xn', name='xn')
                    xp = fg_.tile([128, 3, BT], F32R, tag='xp', name='xp')
                    gt = fg_.tile([128, 3, BT], F32, tag='gt', name='gt')
                    for c in range(3):
                        ps_pe = psp.tile([128, TT], F32, tag='ps', name='ps_pe')
                        for kc in range(2):
                            nc.tensor.matmul(ps_pe[:, :BT], pew_t[:, c, kc, :], patg[:, c, kc, :],
                                             start=(kc == 0), stop=(kc == 1))
                        nc.scalar.activation(ximg[:, c, :], ps_pe[:, :BT], AF.Identity,
                                             bias=fbias_t[:, c:c + 1])
                    xi = [ximg[:, c, :].bitcast(F32) for c in range(3)]
                    st, m, rsd = ln_stats(xi, BT)
                    ln_apply(st, m, rsd, xi, [xn[:, c, :] for c in range(3)], BT)
                    for mc in range(3):
                        ps_ph = psp.tile([128, TT], F32, tag='ps', name='ps_ph')
                        nc.tensor.matmul(ps_ph[:, :BT], phw_t[:, mc, :], pft_t[:, sl],
                                         start=True, stop=True)
                        nc.scalar.activation(xp[:, mc, :], ps_ph[:, :BT], AF.Identity,
                                             bias=fbias_t[:, 3 + mc:4 + mc])
                    for mc in range(3):
                        ps_g = psp.tile([128, TT], F32, tag='ps', name='ps_g')
                        for kc in range(6):
                            rhs = xn[:, kc, :] if kc < 3 else xp[:, kc - 3, :]
                            nc.tensor.matmul(ps_g[:, :BT], gw_t[:, kc, mc, :], rhs,
                                             start=(kc == 0), stop=(kc == 5))
                        nc.scalar.activation(gt[:, mc, :], ps_g[:, :BT], AF.Sigmoid,
                                             bias=fbias_t[:, 6 + mc:7 + mc])
                    for bl in range(2):
                        b = 2 * grp + bl
                        psl = slice(bl * NP_, (bl + 1) * NP_)
                        tsl = slice(b * S + 1, (b + 1) * S)
                        dd = stp.tile([128, 6, TT], F32, tag='st', name='fd')
                        dv = dd[:, 0:3, :NP_]
                        nc.vector.tensor_sub(dv, xn[:, :, psl].bitcast(F32), xp[:, :, psl].bitcast(F32))
                        nc.vector.tensor_mul(dv, gt[:, :, psl], dv)
                        nc.vector.tensor_add(dv, dv, xp[:, :, psl].bitcast(F32))
                        nc.vector.tensor_add(hT[:, :, tsl], dv, pet_t[:])
                        nc.vector.tensor_copy(hT[:, :, b * S:b * S + 1],
                                              fbias_t[:, 9:12].unsqueeze(2))

            # ========================= transformer layers ===================
            with (
                tc.tile_pool(name='wbig', bufs=3) as wb,
                tc.tile_pool(name='wps', bufs=1) as wps,
                tc.tile_pool(name='fgp', bufs=BUFS_FG) as fgp,
                tc.tile_pool(name='midp', bufs=BUFS_MID) as midp,
                tc.tile_pool(name='h2p', bufs=BUFS_H2) as h2p,
                tc.tile_pool(name='amp', bufs=1) as amp,
            ):
                pending = None
                for l in range(n_layers):
                    w1_t = wb.tile([128, 3, 1536], BF16, tag='w', name='w1_t')
                    nc.sync.dma_start(w1_t[:], W1d[l])
                    w2_t = wb.tile([128, 12, 3, 128], BF16, tag='w', name='w2_t')
                    nc.sync.dma_start(w2_t[:], W2d[l])
                    bd_t = wps.tile([128, 3, 4, 128], F32R, tag='bd', name='bd_t')
                    nc.sync.dma_start(bd_t[:], BDd[l])
                    aw1_t = wps.tile([128, 3, 384], F32, tag='aw1', name='aw1_t')
                    nc.sync.dma_start(aw1_t[:], AW1d[l])
                    aw2_t = wps.tile([128, 3, 1024], F32, tag='aw2', name='aw2_t')
                    nc.sync.dma_start(aw2_t[:], AW2d[l])
                    bias_t = wps.tile([128, 26], F32, tag='bias', name='bias_t')
                    nc.sync.dma_start(bias_t[:], BIASd[l])
                    ab2r_t = wps.tile([1, 1024], F32, tag='ab2r', name='ab2r_t')
                    nc.sync.dma_start(ab2r_t[:], AB2Rd[l])
                    b2r_t = wps.tile([1, 3, 128], BF16, tag='b2r', name='b2r_t')
                    nc.sync.dma_start(b2r_t[:], B2Rd[l])

                    hn = hnp.tile([128, 3, NTOK], F32R, tag='hn', name='hn')
                    mh = amp.tile([128, 3, BC], F32, tag='mh', name='mh')
                    if pending is None:
                        sts = []
                        for t in range(4):
                            sl = slice(t * TT, (t + 1) * TT)
                            hs = [hT[:, c, sl] for c in range(3)]
                            sts.append((sl, hs) + ln_stats(hs, TT, cgl=g['cg'][l]))
                    else:
                        sts = pending
                    ps_u = psp.tile([128, TT], F32, tag='ps', name='ps_u')
                    for t in range(4):
                        sl, hs, st, m, rsd = sts[t]
                        ln_apply(st, m, rsd, hs, [hn[:, c, sl] for c in range(3)], TT)
                        bsl = slice(2 * t, 2 * t + 2)
                        for j in range(2):
                            b = 2 * t + j
                            for c in range(3):
                                eng_r = nc.vector
                                eng_r.reduce_sum(mh[:, c, b:b + 1],
                                                 hn[:, c, sl][:, j * S:(j + 1) * S].bitcast(F32),
                                                 axis=mybir.AxisListType.X)
                        for mc in range(3):
                            for kc in range(3):
                                nc.tensor.matmul(
                                    ps_u[:, mc * BC:mc * BC + BC][:, bsl],
                                    aw1_t[:, kc, mc * 128:(mc + 1) * 128],
                                    mh[:, kc, bsl], start=(kc == 0), stop=(kc == 2))

                    u2t = amp.tile([128, 3, BC], F32, tag='u2', name='u2t')
                    for mc in range(3):
                        nc.scalar.activation(u2t[:, mc, :], ps_u[:, mc * BC:mc * BC + BC],
                                             AF.Gelu, bias=bias_t[:, mc:mc + 1])
                    eff = amp.tile([128, 8, BC], F32, tag='eff', name='eff')
                    for mt in range(8):
                        ps_e = psp.tile([128, TT], F32, tag='ps', name='ps_e')
                        for kc in range(3):
                            nc.tensor.matmul(ps_e[:, :BC], aw2_t[:, kc, mt * 128:(mt + 1) * 128],
                                             u2t[:, kc, :], start=(kc == 0), stop=False)
                        nc.tensor.matmul(ps_e[:, :BC], ab2r_t[:, mt * 128:(mt + 1) * 128],
                                         onesf_t[0:1, :BC], start=False, stop=True)
                        nc.vector.tensor_scalar(eff[:, mt, :], ps_e[:, :BC], 1.0, None, ALU.mult)

                    # FFT mixer
                    KCS_F = [[0], [0, 1], [1, 2], [2]]
                    KCS_I = [[0, 1], [1, 2], [2, 3]]
                    for t in range(4):
                        sl = slice(t * TT, (t + 1) * TT)
                        fg = fgp.tile([128, 4, TT], F32R, tag='fg', name='fg')
                        for mc in range(4):
                            ps_F = psp.tile([128, TT], F32, tag='ps', name='ps_F')
                            kcs = KCS_F[mc]
                            for i, kc in enumerate(kcs):
                                nc.tensor.matmul(ps_F[:], bd_t[:, kc, mc, :], hn[:, kc, sl],
                                                 start=(i == 0), stop=(i == len(kcs) - 1))
                            for j in range(2):
                                bb = 2 * t + j
                                nc.scalar.activation(fg[:, mc, j * S:(j + 1) * S],
                                                     ps_F[:, j * S:(j + 1) * S], AF.Gelu,
                                                     scale=eff[:, mc, bb:bb + 1],
                                                     bias=eff[:, 4 + mc, bb:bb + 1])
                        for mc in range(3):
                            ps_A = psp.tile([128, TT], F32, tag='ps', name='ps_A')
                            kcs = KCS_I[mc]
                            for i, kc in enumerate(kcs):
                                nc.tensor.matmul(ps_A[:], ibd_t[:, kc, mc, :], fg[:, kc, :],
                                                 start=(i == 0), stop=(i == len(kcs) - 1))
                            nc.vector.tensor_add(hT[:, mc, sl], hT[:, mc, sl], ps_A[:])

                    # LN2 + MLP (stats pipelined one tile ahead); LN1 stats of
                    # the NEXT layer are emitted per tile right after its
                    # residual lands, overlapping this layer's MLP matmuls.
                    ln2q = []
                    for t in range(4):
                        sl = slice(t * TT, (t + 1) * TT)
                        hs = [hT[:, c, sl] for c in range(3)]
                        ln2q.append((sl, hs) + ln_stats(hs, TT))
                    nxt = []
                    for t in range(4):
                        sl, hs, st, m, rsd = ln2q[t]
                        h2 = h2p.tile([128, 3, TT], BF16, tag='h2', name='h2')
                        ln_apply(st, m, rsd, hs, [h2[:, c, :] for c in range(3)], TT)

                        mid = midp.tile([128, 12, TT], BF16, tag='mid', name='mid')
                        for grp in range(3):
                            pss = []
                            for mci in range(4):
                                mc = grp * 4 + mci
                                ps_m = psp.tile([128, TT], F32, tag='ps', name='ps_m')
                                for kc in range(3):
                                    nc.tensor.matmul(ps_m[:], w1_t[:, kc, mc * 128:(mc + 1) * 128],
                                                     h2[:, kc, :], start=(kc == 0), stop=(kc == 2))
                                pss.append((mc, ps_m))
                            for mc, ps_m in pss:
                                nc.scalar.activation(mid[:, mc, :], ps_m[:], AF.Gelu,
                                                     bias=bias_t[:, 11 + mc:12 + mc])
                        for mc in range(3):
                            ps_o = psp.tile([128, TT], F32, tag='ps', name='ps_o')
                            for kc in range(12):
                                nc.tensor.matmul(ps_o[:], w2_t[:, kc, mc, :], mid[:, kc, :],
                                                 start=(kc == 0), stop=False)
                            nc.tensor.matmul(ps_o[:], b2r_t[:, mc, :], onesb_t[0:1, :TT],
                                             start=False, stop=True)
                            nc.vector.tensor_add(hT[:, mc, sl], hT[:, mc, sl], ps_o[:])
                        if l + 1 < n_layers:
                            nxt.append((sl, hs) + ln_stats(hs, TT, cgl=g['cg'][l + 1]))
                    pending = nxt if l + 1 < n_layers else None

                out_ap = hT[:].rearrange("p c (b s) -> p c b s", s=S)[:, :, :, 0]
                nc.sync.dma_start(HCLS[:], out_ap)

    nc.compile()
    return nc


def _gelu_np(x):
    try:
        from scipy.special import erf
    except ImportError:
        import math
        erf = np.vectorize(math.erf)
    return x * 0.5 * (1.0 + erf(x / np.sqrt(2.0)))


def _head(hcls, g):
    x = hcls.astype(np.float64).T
    m = x.mean(1, keepdims=True)
    v = ((x - m) ** 2).mean(1, keepdims=True)
    cls = (x - m) / np.sqrt(v + EPS) * g['norm_g'] + g['norm_b']
    u = _gelu_np(cls @ g['head_w1'] + g['head_b1'])
    return ((u @ g['head_w2'])[:, 0] + g['head_b2'][0]).astype(np.float32)


def _in_maps(inputs, g):
    x = np.ascontiguousarray(inputs['x'], np.float32)
    pf = np.ascontiguousarray(inputs['patch_feats'], np.float32)
    shared = dict(
        w1=g['W1'], w2=g['W2'], bd=g['BD'], ibd=g['IBD'], aw1=g['AW1'],
        aw2=g['AW2'], bias=g['BIAS'], ab2r=g['AB2R'], b2r=g['B2R'],
        onesf=np.ones((1, BC), np.float32),
        onesb=_bf16(np.ones((1, TT))), pew=g['PEW'], phw=g['PHW'], gw=g['GW'],
        fbias=g['FBIAS'], pet=g['PET'],
        ones=np.ones((128, 128), np.float32),
    )
    Hp = 224 // P
    pat = x.reshape(B, 3, Hp, P, Hp, P).transpose(0, 1, 2, 4, 3, 5).reshape(B, 3, NP_, 2, 128)
    maps = []
    for i in range(NCORES):
        m = dict(shared)
        pc = pat[i * BC:(i + 1) * BC]                       # [BC,3,196,2,128]
        m['patt'] = np.ascontiguousarray(pc.transpose(4, 1, 3, 0, 2).reshape(128, 3, 2, NBP))
        m['pft'] = np.ascontiguousarray(pf[i * BC:(i + 1) * BC].reshape(NBP, 6).T)
        maps.append(m)
    return maps


def kernel(**inputs):
    inputs = {k: np.asarray(v) for k, v in inputs.items()}
    g = _prep(inputs)
    # program structure bakes per-layer ln1 gains into immediates; key on them
    key = (tuple(np.round(np.asarray(g['cg'], np.float64), 12)),)
    if _CACHE.get('key') != key:
        _CACHE['prog'] = _build(g)
        _CACHE['key'] = key
    nc = _CACHE['prog']
    res = run_bass_kernel_spmd(nc, _in_maps(inputs, g), list(range(NCORES)))
    _CACHE['last_res'] = res
    _CACHE['last_g'] = g
    hcls = np.concatenate(
        [r['hcls'].transpose(1, 0, 2).reshape(D, BC) for r in res.results], axis=1)
    return _head(hcls, g)


if __name__ == '__main__':
    d = np.load('/root/problem/ref_data.npz')
    inputs = {k: d[k] for k in d.files if k != 'expected'}
    y = kernel(**inputs)
    exp = d['expected']
    err = np.abs(y - exp)
    print("max abs err:", err.max())
    print("Relative error:", err.max() / np.abs(exp).max())


# revision 30
# speedup vs baseline: 1.0054x; 1.0054x over previous
"""Trainium2 Bass kernel for nn_FFTPermeabilityPredictorPatchPhysics.

Sharding: pure data parallel — 8 samples per NeuronCore, weights replicated.
On-device layout: residual stream transposed, hT [3x128 d-chunks, 1576 tok],
kept in SBUF for all 12 layers. FFT/iFFT as block-diagonal matmuls over a
512-row padded frequency layout (head h -> rows 64h+32s+f). Matmuls run
float32r (full PE rate, ~11-bit mantissa) except the MLP which runs bf16.
LN stats via ones-matmul partition reductions broadcast to all partitions;
the adaptive spectral filter is fused into the ACT-engine gelu via
per-partition scale/bias. All weight folding done host-side in numpy:
double-LN collapse, pre_g/ln2_g into following matmuls, base_filter and
(1+ap) into amlp_w2, 1/197 token-mean into amlp_w1, DFT matrices baked.
Final LN + head on the 64 cls vectors runs host-side in float64.
"""
import numpy as np

import concourse.bacc as bacc
import concourse.mybir as mybir
import concourse.tile as tile
from concourse.bass_utils import run_bass_kernel_spmd

F32 = mybir.dt.float32
F32R = mybir.dt.float32r
BF16 = mybir.dt.bfloat16
AF = mybir.ActivationFunctionType
ALU = mybir.AluOpType

B, D, H, HD, FB, S, L, P, NP_ = 64, 384, 8, 48, 25, 197, 12, 16, 196
EPS = 1e-5
FR = 512
NCORES = 8
BC = B // NCORES     # 8 samples/core
NTOK = BC * S        # 1576
TT = 394             # token tile = 2 samples
NBP = BC * NP_       # 1568
BT = 392             # patch tile = 2 samples

_CACHE = {}
BUFS_HR = 8
BUFS_ST = 4
BUFS_FG = 2
BUFS_MID = 2
BUFS_H2 = 4


def _build_dft():
    n = np.arange(HD)
    k = np.arange(FB)
    ang = -2 * np.pi * np.outer(n, k) / HD
    Cr = np.cos(ang) / np.sqrt(HD)
    Ci = np.sin(ang) / np.sqrt(HD)
    A = np.zeros((FB, HD))
    Bm = np.zeros((FB, HD))
    ifft_w = np.exp(2j * np.pi * np.outer(np.arange(HD), np.arange(HD)) / HD) / np.sqrt(HD)
    for j in range(FB):
        fr = np.zeros(HD, complex)
        fi = np.zeros(HD, complex)
        fr[j] = 1.0
        fi[j] = 1.0j
        if 0 < j < HD - FB + 1:
            fr[HD - j] = 1.0
            fi[HD - j] = -1.0j
        A[j] = (ifft_w @ fr).real
        Bm[j] = (ifft_w @ fi).real
    return Cr, Ci, A, Bm


def _prep(inp, n_layers=L):
    f = {k: np.asarray(v, np.float64) for k, v in inp.items()}
    Cr, Ci, A, Bm = _build_dft()

    BDb = np.zeros((D, FR))
    iBD = np.zeros((FR, D))
    for h in range(H):
        BDb[48 * h:48 * h + 48, 64 * h:64 * h + FB] = Cr
        BDb[48 * h:48 * h + 48, 64 * h + 32:64 * h + 32 + FB] = Ci
        iBD[64 * h:64 * h + FB, 48 * h:48 * h + 48] = A
        iBD[64 * h + 32:64 * h + 32 + FB, 48 * h:48 * h + 48] = Bm

    cg = f['ln1_g'].mean(1)
    assert np.abs(f['ln1_g'] - cg[:, None]).max() < 1e-12, "ln1_g must be constant/layer"
    assert np.abs(f['ln1_b'] - f['ln1_b'].mean(1)[:, None]).max() < 1e-12
    assert np.allclose(f['pe_ln_g'], 1.0) and np.allclose(f['pe_ln_b'], 0.0), "pe_ln fold"

    BD_l = np.einsum('ld,df->ldf', cg[:, None] * f['pre_g'], BDb)
    bdbias_l = np.einsum('ld,df->lf', f['pre_b'], BDb)

    aw1p = np.einsum('ld,lde->lde', cg[:, None] * f['pre_g'], f['amlp_w1']) / S
    ab1p = np.einsum('ld,lde->le', f['pre_b'], f['amlp_w1']) + f['amlp_b1']

    aw2pp = np.zeros((L, D, 2 * FR))
    ab2pp = np.zeros((L, 2 * FR))
    aw2, ab2 = f['amlp_w2'], f['amlp_b2']
    bf, bb = f['base_filter'], f['base_bias']
    for h in range(H):
        for s in range(2):
            for fq in range(FB):
                r = 64 * h + 32 * s + fq
                c0 = h * (FB * 2) + fq * 2
                wf = bf[:, h, fq][:, None] * aw2[:, :, c0]
                bf_ = bf[:, h, fq] * ab2[:, c0] + bf[:, h, fq]
                aw2pp[:, :, r] = wf
                ab2pp[:, r] = bf_
                aw2pp[:, :, FR + r] = bdbias_l[:, r][:, None] * wf
                ab2pp[:, FR + r] = bdbias_l[:, r] * bf_
                if s == 0:
                    aw2pp[:, :, FR + r] += aw2[:, :, c0 + 1]
                    ab2pp[:, FR + r] += bb[:, h, fq] + ab2[:, c0 + 1]

    w1p = np.einsum('ld,lde->lde', f['ln2_g'], f['mlp_w1'])
    b1p = np.einsum('ld,lde->le', f['ln2_b'], f['mlp_w1']) + f['mlp_b1']

    a32 = lambda x: np.ascontiguousarray(x, np.float32)
    g = {}
    g['cg'] = cg
    g['W1'] = _bf16(w1p.reshape(L, 3, 128, 4 * D).transpose(0, 2, 1, 3))            # [L,128,3,1536] bf16
    g['W2'] = _bf16(f['mlp_w2'].reshape(L, 12, 128, 3, 128).transpose(0, 2, 1, 3, 4))
    g['BD'] = a32(BD_l.reshape(L, 3, 128, 4, 128).transpose(0, 2, 1, 3, 4))
    g['IBD'] = a32(iBD.reshape(4, 128, 3, 128).transpose(1, 0, 2, 3))
    g['AW1'] = a32(aw1p.reshape(L, 3, 128, D).transpose(0, 2, 1, 3))
    g['AB2R'] = a32(ab2pp[:, None, :])                                            # [L,1,1024]
    g['B2R'] = _bf16(f['mlp_b2'][:, None, :].reshape(L, 1, 3, 128))
    g['AW2'] = a32(aw2pp.reshape(L, 3, 128, 2 * FR).transpose(0, 2, 1, 3))
    # packed per-layer biases [L,128,26]: 0-2 ab1, 3-10 ab2, 11-22 b1, 23-25 b2
    bias = np.zeros((L, 128, 26))
    bias[:, :, 0:3] = ab1p.reshape(L, 3, 128).transpose(0, 2, 1)
    bias[:, :, 3:11] = ab2pp.reshape(L, 8, 128).transpose(0, 2, 1)
    bias[:, :, 11:23] = b1p.reshape(L, 12, 128).transpose(0, 2, 1)
    bias[:, :, 23:26] = f['mlp_b2'].reshape(L, 3, 128).transpose(0, 2, 1)
    g['BIAS'] = a32(bias)
    g['PEW'] = a32(f['pe_w'].reshape(3, 2, 128, 128).transpose(2, 0, 1, 3))          # [128,3,2,128]
    g['PHW'] = a32(f['phys_w'].reshape(6, 3, 128))                                   # [6,3,128]
    g['GW'] = a32(f['gate_w'].reshape(6, 128, 3, 128).transpose(1, 0, 2, 3))         # [128,6,3,128]
    fbias = np.zeros((128, 12))  # 0-2 peb, 3-5 phb, 6-8 gb, 9-11 clspe
    fbias[:, 0:3] = f['pe_b'].T
    fbias[:, 3:6] = f['phys_b'].reshape(3, 128).T
    fbias[:, 6:9] = f['gate_b'].reshape(3, 128).T
    fbias[:, 9:12] = (f['cls_token'][0, 0] + f['pos_embed'][0, 0]).reshape(3, 128).T
    g['FBIAS'] = a32(fbias)
    g['PET'] = a32(f['pos_embed'][0, 1:].T.reshape(3, 128, NP_).transpose(1, 0, 2))  # [128,3,196]
    for kk in ('norm_g', 'norm_b', 'head_w1', 'head_b1', 'head_w2', 'head_b2'):
        g[kk] = f[kk]
    g['n_layers'] = n_layers
    return g


def _bf16(x):
    import ml_dtypes
    return np.ascontiguousarray(np.asarray(x, np.float32), dtype=ml_dtypes.bfloat16)


def _build(g):
    n_layers = g['n_layers']
    nc = bacc.Bacc('TRN2', target_bir_lowering=False, debug=False)
    # register float constants used as ACT biases
    for val in (EPS,):
        t = nc.alloc_sbuf_tensor(f"const-f32-{val}", [128, 1], F32)
        nc.gpsimd.memset(t.ap(), val)
        nc.const_aps.aps[(F32, val)] = t.ap()
    nc.all_engine_barrier()

    di = lambda name, shape, dt: nc.dram_tensor(name, list(shape), dt, kind="ExternalInput")
    PATd = di('patt', (128, 3, 2, NBP), F32R)
    PFT = di('pft', (6, NBP), F32R)
    W1d = di('w1', (L, 128, 3, 1536), BF16)
    W2d = di('w2', (L, 128, 12, 3, 128), BF16)
    BDd = di('bd', (L, 128, 3, 4, 128), F32R)
    IBDd = di('ibd', (128, 4, 3, 128), F32R)
    AW1d = di('aw1', (L, 128, 3, 384), F32)
    AW2d = di('aw2', (L, 128, 3, 1024), F32)
    BIASd = di('bias', (L, 128, 26), F32)
    AB2Rd = di('ab2r', (L, 1, 1024), F32)
    B2Rd = di('b2r', (L, 1, 3, 128), BF16)
    ONFd = di('onesf', (1, BC), F32)
    ONBd = di('onesb', (1, TT), BF16)
    PEWd = di('pew', (128, 3, 2, 128), F32R)
    PHWd = di('phw', (6, 3, 128), F32R)
    GWd = di('gw', (128, 6, 3, 128), F32R)
    FBIASd = di('fbias', (128, 12), F32)
    PETd = di('pet', (128, 3, NP_), F32)
    ONESd = di('ones', (128, 128), F32R)
    HCLS = nc.dram_tensor('hcls', [128, 3, BC], F32, kind="ExternalOutput")

    with tile.TileContext(nc) as tc:
        with (
            tc.tile_pool(name='const', bufs=1) as cp,
            tc.tile_pool(name='persist', bufs=1) as pp,
            tc.tile_pool(name='hnp', bufs=1) as hnp,
            tc.tile_pool(name='hrp', bufs=BUFS_HR) as hrp,
            tc.tile_pool(name='stp', bufs=BUFS_ST) as stp,
            tc.tile_pool(name='psp', bufs=8, space='PSUM') as psp,
        ):
            ones_t = cp.tile([128, 128], F32R, name='ones_t')
            nc.sync.dma_start(ones_t[:], ONESd[:])
            ibd_t = cp.tile([128, 4, 3, 128], F32R, name='ibd_t')
            nc.sync.dma_start(ibd_t[:], IBDd[:])
            onesf_t = cp.tile([1, BC], F32, name='onesf_t')
            nc.sync.dma_start(onesf_t[:], ONFd[:])
            onesb_t = cp.tile([1, TT], BF16, name='onesb_t')
            nc.sync.dma_start(onesb_t[:], ONBd[:])
            fbias_t = cp.tile([128, 12], F32, name='fbias_t')
            nc.sync.dma_start(fbias_t[:], FBIASd[:])
            pet_t = cp.tile([128, 3, NP_], F32, name='pet_t')
            nc.sync.dma_start(pet_t[:], PETd[:])

            hT = pp.tile([128, 3, NTOK], F32, name='hT')

            def ln_stats(srcs, tlen, cgl=None, pstag='ps'):
                """LN stats for one token tile; srcs = 3 [128,tlen] f32 APs.
                Double-LN folds to a single rsqrt:
                rs1*rs2 = rsqrt((cg^2+eps)*v + eps^2). Returns (st, m, rsd)."""
                hrs = []
                for c in range(3):
                    hr = hrp.tile([128, TT], F32R, tag='hr', name='hr')
                    xq = hrp.tile([128, TT], F32R, tag='xq', name='xq')
                    nc.gpsimd.tensor_scalar(hr[:, :tlen], srcs[c], 1.0, None, ALU.mult)
                    eng_q = nc.gpsimd if c == 2 else nc.vector
                    eng_q.tensor_mul(xq[:, :tlen], srcs[c], srcs[c])
                    hrs.append((hr, xq))
                ps_s = psp.tile([128, TT], F32, tag=pstag, name='ps_s')
                ps_q = psp.tile([128, TT], F32, tag=pstag, name='ps_q')
                for c in range(3):
                    nc.tensor.matmul(ps_s[:, :tlen], ones_t[:], hrs[c][0][:, :tlen],
                                     start=(c == 0), stop=(c == 2))
                for c in range(3):
                    nc.tensor.matmul(ps_q[:, :tlen], ones_t[:], hrs[c][1][:, :tlen],
                                     start=(c == 0), stop=(c == 2))
                if cgl is None:
                    A, Bc_ = 1.0, EPS
                else:
                    A = float(cgl) * float(cgl) + EPS
                    Bc_ = EPS * EPS
                import math
                sA = math.sqrt(A)
                st = stp.tile([128, 6, TT], F32, tag='st', name='st')
                m = st[:, 0, :tlen]
                mA = st[:, 1, :tlen]
                msqA = st[:, 2, :tlen]
                t1 = st[:, 3, :tlen]
                ve = st[:, 4, :tlen]
                u = st[:, 5, :tlen]
                rsd = st[:, 1, :tlen]   # mA dead after msqA
                nc.vector.tensor_scalar(m, ps_s[:, :tlen], 1.0 / D, None, ALU.mult)
                nc.vector.tensor_scalar(mA, ps_s[:, :tlen], sA / D, None, ALU.mult)
                nc.vector.tensor_mul(msqA, mA, mA)
                nc.vector.tensor_scalar(t1, ps_q[:, :tlen], A / D, Bc_, ALU.mult, ALU.add)
                nc.vector.tensor_sub(ve, t1, msqA)
                nc.vector.reciprocal(u, ve)
                nc.scalar.activation(rsd, u, AF.Sqrt)
                return st, m, rsd

            def ln_apply(st, m, rsd, srcs, dsts, tlen):
                tmp = st[:, 2, :tlen]
                tmp2 = st[:, 3, :tlen]
                for c in range(3):
                    eng = nc.gpsimd if c == 2 else nc.vector
                    tm = tmp2 if c == 2 else tmp
                    eng.tensor_sub(tm, srcs[c], m)
                    eng.tensor_mul(dsts[c], tm, rsd)

            # ================= front (streamed per 2-sample group) ==========
            with (
                tc.tile_pool(name='fgrp', bufs=2) as fg_,
                tc.tile_pool(name='fw', bufs=1) as fw,
            ):
                pft_t = fw.tile([6, NBP], F32R, name='pft_t')
                nc.sync.dma_start(pft_t[:], PFT[:])
                pew_t = fw.tile([128, 3, 2, 128], F32R, name='pew_t')
                nc.sync.dma_start(pew_t[:], PEWd[:])
                phw_t = fw.tile([6, 3, 128], F32R, name='phw_t')
                nc.sync.dma_start(phw_t[:], PHWd[:])
                for grp in range(4):
                    sl = slice(grp * BT, (grp + 1) * BT)
                    patg = fg_.tile([128, 3, 2, BT], F32R, tag='patg', name='patg')
                    for c in range(3):
                        nc.sync.dma_start(patg[:, c], PATd[:, c, :, sl])
                    ximg = fg_.tile([128, 3, BT], F32R, tag='ximg', name='ximg')
                    xn = fg_.tile([128, 3, BT], F32R, tag='xn', name='xn')
                    xp = fg_.tile([128, 3, BT], F32R, tag='xp', name='xp')
                    gt = fg_.tile([128, 3, BT], F32, tag='gt', name='gt')
                    for c in range(3):
                        ps_pe = psp.tile([128, TT], F32, tag='ps', name='ps_pe')
                        for kc in range(2):
                            nc.tensor.matmul(ps_pe[:, :BT], pew_t[:, c, kc, :], patg[:, c, kc, :],
                                             start=(kc == 0), stop=(kc == 1))
                        nc.scalar.activation(ximg[:, c, :], ps_pe[:, :BT], AF.Identity,
                                             bias=fbias_t[:, c:c + 1])
                    if grp == 0:
                        gw_t = fw.tile([128, 6, 3, 128], F32R, name='gw_t')
                        nc.sync.dma_start(gw_t[:], GWd[:])
                    xi = [ximg[:, c, :].bitcast(F32) for c in range(3)]
                    st, m, rsd = ln_stats(xi, BT)
                    ln_apply(st, m, rsd, xi, [xn[:, c, :] for c in range(3)], BT)
                    for mc in range(3):
                        ps_ph = psp.tile([128, TT], F32, tag='ps', name='ps_ph')
                        nc.tensor.matmul(ps_ph[:, :BT], phw_t[:, mc, :], pft_t[:, sl],
                                         start=True, stop=True)
                        nc.scalar.activation(xp[:, mc, :], ps_ph[:, :BT], AF.Identity,
                                             bias=fbias_t[:, 3 + mc:4 + mc])
                    for mc in range(3):
                        ps_g = psp.tile([128, TT], F32, tag='ps', name='ps_g')
                        for kc in range(6):
                            rhs = xn[:, kc, :] if kc < 3 else xp[:, kc - 3, :]
                            nc.tensor.matmul(ps_g[:, :BT], gw_t[:, kc, mc, :], rhs,
                                             start=(kc == 0), stop=(kc == 5))
                        nc.scalar.activation(gt[:, mc, :], ps_g[:, :BT], AF.Sigmoid,
                                             bias=fbias_t[:, 6 + mc:7 + mc])
                    for bl in range(2):
                        b = 2 * grp + bl
                        psl = slice(bl * NP_, (bl + 1) * NP_)
                        tsl = slice(b * S + 1, (b + 1) * S)
                        dd = stp.tile([128, 6, TT], F32, tag='st', name='fd')
                        dv = dd[:, 0:3, :NP_]
                        nc.vector.tensor_sub(dv, xn[:, :, psl].bitcast(F32), xp[:, :, psl].bitcast(F32))
                        nc.vector.tensor_mul(dv, gt[:, :, psl], dv)
                        nc.vector.tensor_add(dv, dv, xp[:, :, psl].bitcast(F32))
                        nc.vector.tensor_add(hT[:, :, tsl], dv, pet_t[:])
                        nc.vector.tensor_copy(hT[:, :, b * S:b * S + 1],
                                              fbias_t[:, 9:12].unsqueeze(2))

            # ========================= transformer layers ===================
            with (
                tc.tile_pool(name='wbig', bufs=3) as wb,
                tc.tile_pool(name='wps', bufs=1) as wps,
                tc.tile_pool(name='fgp', bufs=BUFS_FG) as fgp,
                tc.tile_pool(name='midp', bufs=BUFS_MID) as midp,
                tc.tile_pool(name='h2p', bufs=BUFS_H2) as h2p,
                tc.tile_pool(name='amp', bufs=1) as amp,
            ):
                pending = None
                for l in range(n_layers):
                    w1_t = wb.tile([128, 3, 1536], BF16, tag='w', name='w1_t')
                    nc.sync.dma_start(w1_t[:], W1d[l])
                    w2_t = wb.tile([128, 12, 3, 128], BF16, tag='w', name='w2_t')
                    nc.sync.dma_start(w2_t[:], W2d[l])
                    bd_t = wps.tile([128, 3, 4, 128], F32R, tag='bd', name='bd_t')
                    nc.sync.dma_start(bd_t[:], BDd[l])
                    aw1_t = wps.tile([128, 3, 384], F32, tag='aw1', name='aw1_t')
                    nc.sync.dma_start(aw1_t[:], AW1d[l])
                    aw2_t = wps.tile([128, 3, 1024], F32, tag='aw2', name='aw2_t')
                    nc.sync.dma_start(aw2_t[:], AW2d[l])
                    bias_t = wps.tile([128, 26], F32, tag='bias', name='bias_t')
                    nc.sync.dma_start(bias_t[:], BIASd[l])
                    ab2r_t = wps.tile([1, 1024], F32, tag='ab2r', name='ab2r_t')
                    nc.sync.dma_start(ab2r_t[:], AB2Rd[l])
                    b2r_t = wps.tile([1, 3, 128], BF16, tag='b2r', name='b2r_t')
                    nc.sync.dma_start(b2r_t[:], B2Rd[l])

                    hn = hnp.tile([128, 3, NTOK], F32R, tag='hn', name='hn')
                    mh = amp.tile([128, 3, BC], F32, tag='mh', name='mh')
                    if pending is None:
                        sts = []
                        for t in range(4):
                            sl = slice(t * TT, (t + 1) * TT)
                            hs = [hT[:, c, sl] for c in range(3)]
                            sts.append((sl, hs) + ln_stats(hs, TT, cgl=g['cg'][l]))
                    else:
                        sts = pending
                    ps_u = psp.tile([128, TT], F32, tag='ps', name='ps_u')
                    for t in range(4):
                        sl, hs, st, m, rsd = sts[t]
                        ln_apply(st, m, rsd, hs, [hn[:, c, sl] for c in range(3)], TT)
                        bsl = slice(2 * t, 2 * t + 2)
                        for j in range(2):
                            b = 2 * t + j
                            for c in range(3):
                                eng_r = nc.vector
                                eng_r.reduce_sum(mh[:, c, b:b + 1],
                                                 hn[:, c, sl][:, j * S:(j + 1) * S].bitcast(F32),
                                                 axis=mybir.AxisListType.X)
                        for mc in range(3):
                            for kc in range(3):
                                nc.tensor.matmul(
                                    ps_u[:, mc * BC:mc * BC + BC][:, bsl],
                                    aw1_t[:, kc, mc * 128:(mc + 1) * 128],
                                    mh[:, kc, bsl], start=(kc == 0), stop=(kc == 2))

                    u2t = amp.tile([128, 3, BC], F32, tag='u2', name='u2t')
                    for mc in range(3):
                        nc.scalar.activation(u2t[:, mc, :], ps_u[:, mc * BC:mc * BC + BC],
                                             AF.Gelu, bias=bias_t[:, mc:mc + 1])
                    eff = amp.tile([128, 8, BC], F32, tag='eff', name='eff')
                    for mt in range(8):
                        ps_e = psp.tile([128, TT], F32, tag='ps', name='ps_e')
                        for kc in range(3):
                            nc.tensor.matmul(ps_e[:, :BC], aw2_t[:, kc, mt * 128:(mt + 1) * 128],
                                             u2t[:, kc, :], start=(kc == 0), stop=False)
                        nc.tensor.matmul(ps_e[:, :BC], ab2r_t[:, mt * 128:(mt + 1) * 128],
                                         onesf_t[0:1, :BC], start=False, stop=True)
                        nc.vector.tensor_scalar(eff[:, mt, :], ps_e[:, :BC], 1.0, None, ALU.mult)

                    # FFT mixer
                    KCS_F = [[0], [0, 1], [1, 2], [2]]
                    KCS_I = [[0, 1], [1, 2], [2, 3]]
                    for t in range(4):
                        sl = slice(t * TT, (t + 1) * TT)
                        fg = fgp.tile([128, 4, TT], F32R, tag='fg', name='fg')
                        for mc in range(4):
                            ps_F = psp.tile([128, TT], F32, tag='ps', name='ps_F')
                            kcs = KCS_F[mc]
                            for i, kc in enumerate(kcs):
                                nc.tensor.matmul(ps_F[:], bd_t[:, kc, mc, :], hn[:, kc, sl],
                                                 start=(i == 0), stop=(i == len(kcs) - 1))
                            for j in range(2):
                                bb = 2 * t + j
                                nc.scalar.activation(fg[:, mc, j * S:(j + 1) * S],
                                                     ps_F[:, j * S:(j + 1) * S], AF.Gelu,
                                                     scale=eff[:, mc, bb:bb + 1],
                                                     bias=eff[:, 4 + mc, bb:bb + 1])
                        for mc in range(3):
                            ps_A = psp.tile([128, TT], F32, tag='ps', name='ps_A')
                            kcs = KCS_I[mc]
                            for i, kc in enumerate(kcs):
                                nc.tensor.matmul(ps_A[:], ibd_t[:, kc, mc, :], fg[:, kc, :],
                                                 start=(i == 0), stop=(i == len(kcs) - 1))
                            nc.vector.tensor_add(hT[:, mc, sl], hT[:, mc, sl], ps_A[:])

                    # LN2 + MLP (stats pipelined one tile ahead); LN1 stats of
                    # the NEXT layer are emitted per tile right after its
                    # residual lands, overlapping this layer's MLP matmuls.
                    ln2q = []
                    for t in range(4):
                        sl = slice(t * TT, (t + 1) * TT)
                        hs = [hT[:, c, sl] for c in range(3)]
                        ln2q.append((sl, hs) + ln_stats(hs, TT))
                    nxt = []
                    for t in range(4):
                        sl, hs, st, m, rsd = ln2q[t]
                        h2 = h2p.tile([128, 3, TT], BF16, tag='h2', name='h2')
                        ln_apply(st, m, rsd, hs, [h2[:, c, :] for c in range(3)], TT)

                        mid = midp.tile([128, 12, TT], BF16, tag='mid', name='mid')
                        for grp in range(3):
                            pss = []
                            for mci in range(4):
                                mc = grp * 4 + mci
                                ps_m = psp.tile([128, TT], F32, tag='ps', name='ps_m')
                                for kc in range(3):
                                    nc.tensor.matmul(ps_m[:], w1_t[:, kc, mc * 128:(mc + 1) * 128],
                                                     h2[:, kc, :], start=(kc == 0), stop=(kc == 2))
                                pss.append((mc, ps_m))
                            for mc, ps_m in pss:
                                nc.scalar.activation(mid[:, mc, :], ps_m[:], AF.Gelu,
                                                     bias=bias_t[:, 11 + mc:12 + mc])
                        for mc in range(3):
                            ps_o = psp.tile([128, TT], F32, tag='ps', name='ps_o')
                            for kc in range(12):
                                nc.tensor.matmul(ps_o[:], w2_t[:, kc, mc, :], mid[:, kc, :],
                                                 start=(kc == 0), stop=False)
                            nc.tensor.matmul(ps_o[:], b2r_t[:, mc, :], onesb_t[0:1, :TT],
                                             start=False, stop=True)
                            nc.vector.tensor_add(hT[:, mc, sl], hT[:, mc, sl], ps_o[:])
                        if l + 1 < n_layers:
                            nxt.append((sl, hs) + ln_stats(hs, TT, cgl=g['cg'][l + 1]))
                    pending = nxt if l + 1 < n_layers else None

                out_ap = hT[:].rearrange("p c (b s) -> p c b s", s=S)[:, :, :, 0]
                nc.sync.dma_start(HCLS[:], out_ap)

    nc.compile()
    return nc


def _gelu_np(x):
    try:
        from scipy.special import erf
    except ImportError:
        import math
        erf = np.vectorize(math.erf)
    return x * 0.5 * (1.0 + erf(x / np.sqrt(2.0)))


def _head(hcls, g):
    x = hcls.astype(np.float64).T
    m = x.mean(1, keepdims=True)
    v = ((x - m) ** 2).mean(1, keepdims=True)
    cls = (x - m) / np.sqrt(v + EPS) * g['norm_g'] + g['norm_b']
    u = _gelu_np(cls @ g['head_w1'] + g['head_b1'])
    return ((u @ g['head_w2'])[:, 0] + g['head_b2'][0]).astype(np.float32)


def _in_maps(inputs, g):
    x = np.ascontiguousarray(inputs['x'], np.float32)
    pf = np.ascontiguousarray(inputs['patch_feats'], np.float32)
    shared = dict(
        w1=g['W1'], w2=g['W2'], bd=g['BD'], ibd=g['IBD'], aw1=g['AW1'],
        aw2=g['AW2'], bias=g['BIAS'], ab2r=g['AB2R'], b2r=g['B2R'],
        onesf=np.ones((1, BC), np.float32),
        onesb=_bf16(np.ones((1, TT))), pew=g['PEW'], phw=g['PHW'], gw=g['GW'],
        fbias=g['FBIAS'], pet=g['PET'],
        ones=np.ones((128, 128), np.float32),
    )
    Hp = 224 // P
    pat = x.reshape(B, 3, Hp, P, Hp, P).transpose(0, 1, 2, 4, 3, 5).reshape(B, 3, NP_, 2, 128)
    maps = []
    for i in range(NCORES):
        m = dict(shared)
        pc = pat[i * BC:(i + 1) * BC]                       # [BC,3,196,2,128]
        m['patt'] = np.ascontiguousarray(pc.transpose(4, 1, 3, 0, 2).reshape(128, 3, 2, NBP))
        m['pft'] = np.ascontiguousarray(pf[i * BC:(i + 1) * BC].reshape(NBP, 6).T)
        maps.append(m)
    return maps


def kernel(**inputs):
    inputs = {k: np.asarray(v) for k, v in inputs.items()}
    g = _prep(inputs)
    # program structure bakes per-layer ln1 gains into immediates; key on them
    key = (tuple(np.round(np.asarray(g['cg'], np.float64), 12)),)
    if _CACHE.get('key') != key:
        _CACHE['prog'] = _build(g)
        _CACHE['key'] = key
    nc = _CACHE['prog']
    res = run_bass_kernel_spmd(nc, _in_maps(inputs, g), list(range(NCORES)))
    _CACHE['last_res'] = res
    _CACHE['last_g'] = g
    hcls = np.concatenate(
        [r['hcls'].transpose(1, 0, 2).reshape(D, BC) for r in res.results], axis=1)
    return _head(hcls, g)


if __name__ == '__main__':
    d = np.load('/root/problem/ref_data.npz')
    inputs = {k: d[k] for k in d.files if k != 'expected'}
    y = kernel(**inputs)
    exp = d['expected']
    err = np.abs(y - exp)
    print("max abs err:", err.max())
    print("Relative error:", err.max() / np.abs(exp).max())


# revision 33
# speedup vs baseline: 1.0357x; 1.0301x over previous
"""Trainium2 Bass kernel for nn_FFTPermeabilityPredictorPatchPhysics.

Sharding: pure data parallel — 8 samples per NeuronCore, weights replicated.
On-device layout: residual stream transposed, hT [3x128 d-chunks, 1576 tok],
kept in SBUF for all 12 layers. FFT/iFFT as block-diagonal matmuls over a
512-row padded frequency layout (head h -> rows 64h+32s+f). Matmuls run
float32r (full PE rate, ~11-bit mantissa) except the MLP which runs bf16.
LN stats via ones-matmul partition reductions broadcast to all partitions;
the adaptive spectral filter is fused into the ACT-engine gelu via
per-partition scale/bias. All weight folding done host-side in numpy:
double-LN collapse, pre_g/ln2_g into following matmuls, base_filter and
(1+ap) into amlp_w2, 1/197 token-mean into amlp_w1, DFT matrices baked.
Final LN + head on the 64 cls vectors runs host-side in float64.
"""
import numpy as np

import concourse.bacc as bacc
import concourse.mybir as mybir
import concourse.tile as tile
from concourse.bass_utils import run_bass_kernel_spmd

F32 = mybir.dt.float32
F32R = mybir.dt.float32r
BF16 = mybir.dt.bfloat16
AF = mybir.ActivationFunctionType
ALU = mybir.AluOpType

B, D, H, HD, FB, S, L, P, NP_ = 64, 384, 8, 48, 25, 197, 12, 16, 196
EPS = 1e-5
FR = 512
NCORES = 8
BC = B // NCORES     # 8 samples/core
NTOK = BC * S        # 1576
TT = 394             # token tile = 2 samples
NBP = BC * NP_       # 1568
BT = 392             # patch tile = 2 samples

_CACHE = {}
BUFS_HR = 8
BUFS_ST = 4
BUFS_FG = 2
BUFS_MID = 2
BUFS_H2 = 4


def _build_dft():
    n = np.arange(HD)
    k = np.arange(FB)
    ang = -2 * np.pi * np.outer(n, k) / HD
    Cr = np.cos(ang) / np.sqrt(HD)
    Ci = np.sin(ang) / np.sqrt(HD)
    A = np.zeros((FB, HD))
    Bm = np.zeros((FB, HD))
    ifft_w = np.exp(2j * np.pi * np.outer(np.arange(HD), np.arange(HD)) / HD) / np.sqrt(HD)
    for j in range(FB):
        fr = np.zeros(HD, complex)
        fi = np.zeros(HD, complex)
        fr[j] = 1.0
        fi[j] = 1.0j
        if 0 < j < HD - FB + 1:
            fr[HD - j] = 1.0
            fi[HD - j] = -1.0j
        A[j] = (ifft_w @ fr).real
        Bm[j] = (ifft_w @ fi).real
    return Cr, Ci, A, Bm


def _prep(inp, n_layers=L):
    f = {k: np.asarray(v, np.float64) for k, v in inp.items()}
    Cr, Ci, A, Bm = _build_dft()

    BDb = np.zeros((D, FR))
    iBD = np.zeros((FR, D))
    for h in range(H):
        BDb[48 * h:48 * h + 48, 64 * h:64 * h + FB] = Cr
        BDb[48 * h:48 * h + 48, 64 * h + 32:64 * h + 32 + FB] = Ci
        iBD[64 * h:64 * h + FB, 48 * h:48 * h + 48] = A
        iBD[64 * h + 32:64 * h + 32 + FB, 48 * h:48 * h + 48] = Bm

    cg = f['ln1_g'].mean(1)
    assert np.abs(f['ln1_g'] - cg[:, None]).max() < 1e-12, "ln1_g must be constant/layer"
    assert np.abs(f['ln1_b'] - f['ln1_b'].mean(1)[:, None]).max() < 1e-12
    assert np.allclose(f['pe_ln_g'], 1.0) and np.allclose(f['pe_ln_b'], 0.0), "pe_ln fold"

    BD_l = np.einsum('ld,df->ldf', cg[:, None] * f['pre_g'], BDb)
    bdbias_l = np.einsum('ld,df->lf', f['pre_b'], BDb)

    aw1p = np.einsum('ld,lde->lde', cg[:, None] * f['pre_g'], f['amlp_w1']) / S
    ab1p = np.einsum('ld,lde->le', f['pre_b'], f['amlp_w1']) + f['amlp_b1']

    aw2pp = np.zeros((L, D, 2 * FR))
    ab2pp = np.zeros((L, 2 * FR))
    aw2, ab2 = f['amlp_w2'], f['amlp_b2']
    bf, bb = f['base_filter'], f['base_bias']
    for h in range(H):
        for s in range(2):
            for fq in range(FB):
                r = 64 * h + 32 * s + fq
                c0 = h * (FB * 2) + fq * 2
                wf = bf[:, h, fq][:, None] * aw2[:, :, c0]
                bf_ = bf[:, h, fq] * ab2[:, c0] + bf[:, h, fq]
                aw2pp[:, :, r] = wf
                ab2pp[:, r] = bf_
                aw2pp[:, :, FR + r] = bdbias_l[:, r][:, None] * wf
                ab2pp[:, FR + r] = bdbias_l[:, r] * bf_
                if s == 0:
                    aw2pp[:, :, FR + r] += aw2[:, :, c0 + 1]
                    ab2pp[:, FR + r] += bb[:, h, fq] + ab2[:, c0 + 1]

    w1p = np.einsum('ld,lde->lde', f['ln2_g'], f['mlp_w1'])
    b1p = np.einsum('ld,lde->le', f['ln2_b'], f['mlp_w1']) + f['mlp_b1']

    a32 = lambda x: np.ascontiguousarray(x, np.float32)
    g = {}
    g['cg'] = cg
    g['W1'] = _bf16(w1p.reshape(L, 3, 128, 4 * D).transpose(0, 2, 1, 3))            # [L,128,3,1536] bf16
    g['W2'] = _bf16(f['mlp_w2'].reshape(L, 12, 128, 3, 128).transpose(0, 2, 1, 3, 4))
    g['BD'] = a32(BD_l.reshape(L, 3, 128, 4, 128).transpose(0, 2, 1, 3, 4))
    g['IBD'] = a32(iBD.reshape(4, 128, 3, 128).transpose(1, 0, 2, 3))
    g['AW1'] = a32(aw1p.reshape(L, 3, 128, D).transpose(0, 2, 1, 3))
    g['AB2R'] = a32(ab2pp[:, None, :])                                            # [L,1,1024]
    g['B2R'] = _bf16(f['mlp_b2'][:, None, :].reshape(L, 1, 3, 128))
    g['AW2'] = a32(aw2pp.reshape(L, 3, 128, 2 * FR).transpose(0, 2, 1, 3))
    # packed per-layer biases [L,128,26]: 0-2 ab1, 3-10 ab2, 11-22 b1, 23-25 b2
    bias = np.zeros((L, 128, 26))
    bias[:, :, 0:3] = ab1p.reshape(L, 3, 128).transpose(0, 2, 1)
    bias[:, :, 3:11] = ab2pp.reshape(L, 8, 128).transpose(0, 2, 1)
    bias[:, :, 11:23] = b1p.reshape(L, 12, 128).transpose(0, 2, 1)
    bias[:, :, 23:26] = f['mlp_b2'].reshape(L, 3, 128).transpose(0, 2, 1)
    g['BIAS'] = a32(bias)
    g['PEW'] = a32(f['pe_w'].reshape(3, 2, 128, 128).transpose(2, 0, 1, 3))          # [128,3,2,128]
    g['PHW'] = a32(f['phys_w'].reshape(6, 3, 128))                                   # [6,3,128]
    g['GW'] = a32(f['gate_w'].reshape(6, 128, 3, 128).transpose(1, 0, 2, 3))         # [128,6,3,128]
    fbias = np.zeros((128, 12))  # 0-2 peb, 3-5 phb, 6-8 gb, 9-11 clspe
    fbias[:, 0:3] = f['pe_b'].T
    fbias[:, 3:6] = f['phys_b'].reshape(3, 128).T
    fbias[:, 6:9] = f['gate_b'].reshape(3, 128).T
    fbias[:, 9:12] = (f['cls_token'][0, 0] + f['pos_embed'][0, 0]).reshape(3, 128).T
    g['FBIAS'] = a32(fbias)
    g['PET'] = a32(f['pos_embed'][0, 1:].T.reshape(3, 128, NP_).transpose(1, 0, 2))  # [128,3,196]
    for kk in ('norm_g', 'norm_b', 'head_w1', 'head_b1', 'head_w2', 'head_b2'):
        g[kk] = f[kk]
    g['n_layers'] = n_layers
    return g


def _bf16(x):
    import ml_dtypes
    return np.ascontiguousarray(np.asarray(x, np.float32), dtype=ml_dtypes.bfloat16)


def _build(g):
    n_layers = g['n_layers']
    nc = bacc.Bacc('TRN2', target_bir_lowering=False, debug=False)
    # register float constants used as ACT biases
    for val in (EPS,):
        t = nc.alloc_sbuf_tensor(f"const-f32-{val}", [128, 1], F32)
        nc.gpsimd.memset(t.ap(), val)
        nc.const_aps.aps[(F32, val)] = t.ap()
    nc.all_engine_barrier()

    di = lambda name, shape, dt: nc.dram_tensor(name, list(shape), dt, kind="ExternalInput")
    PATd = di('patt', (128, 3, 2, NBP), F32R)
    PFT = di('pft', (6, NBP), F32R)
    W1d = di('w1', (L, 128, 3, 1536), BF16)
    W2d = di('w2', (L, 128, 12, 3, 128), BF16)
    BDd = di('bd', (L, 128, 3, 4, 128), F32R)
    IBDd = di('ibd', (128, 4, 3, 128), F32R)
    AW1d = di('aw1', (L, 128, 3, 384), F32)
    AW2d = di('aw2', (L, 128, 3, 1024), F32)
    BIASd = di('bias', (L, 128, 26), F32)
    AB2Rd = di('ab2r', (L, 1, 1024), F32)
    B2Rd = di('b2r', (L, 1, 3, 128), BF16)
    ONFd = di('onesf', (1, BC), F32)
    ONBd = di('onesb', (1, TT), BF16)
    PEWd = di('pew', (128, 3, 2, 128), F32R)
    PHWd = di('phw', (6, 3, 128), F32R)
    GWd = di('gw', (128, 6, 3, 128), F32R)
    FBIASd = di('fbias', (128, 12), F32)
    PETd = di('pet', (128, 3, NP_), F32)
    ONESd = di('ones', (128, 128), F32R)
    HCLS = nc.dram_tensor('hcls', [128, 3, BC], F32, kind="ExternalOutput")

    with tile.TileContext(nc) as tc:
        with (
            tc.tile_pool(name='const', bufs=1) as cp,
            tc.tile_pool(name='persist', bufs=1) as pp,
            tc.tile_pool(name='hnp', bufs=1) as hnp,
            tc.tile_pool(name='hrp', bufs=BUFS_HR) as hrp,
            tc.tile_pool(name='stp', bufs=BUFS_ST) as stp,
            tc.tile_pool(name='psp', bufs=8, space='PSUM') as psp,
        ):
            ones_t = cp.tile([128, 128], F32R, name='ones_t')
            nc.sync.dma_start(ones_t[:], ONESd[:])
            ibd_t = cp.tile([128, 4, 3, 128], F32R, name='ibd_t')
            nc.sync.dma_start(ibd_t[:], IBDd[:])
            onesf_t = cp.tile([1, BC], F32, name='onesf_t')
            nc.sync.dma_start(onesf_t[:], ONFd[:])
            onesb_t = cp.tile([1, TT], BF16, name='onesb_t')
            nc.sync.dma_start(onesb_t[:], ONBd[:])
            fbias_t = cp.tile([128, 12], F32, name='fbias_t')
            nc.sync.dma_start(fbias_t[:], FBIASd[:])
            pet_t = cp.tile([128, 3, NP_], F32, name='pet_t')
            nc.sync.dma_start(pet_t[:], PETd[:])

            hT = pp.tile([128, 3, NTOK], F32, name='hT')

            def ln_stats(srcs, tlen, cgl=None, pstag='ps'):
                """LN stats for one token tile; srcs = 3 [128,tlen] f32 APs.
                Double-LN folds to a single rsqrt:
                rs1*rs2 = rsqrt((cg^2+eps)*v + eps^2). Returns (st, m, rsd)."""
                hrs = []
                for c in range(3):
                    hr = hrp.tile([128, TT], F32R, tag='hr', name='hr')
                    xq = hrp.tile([128, TT], F32R, tag='xq', name='xq')
                    nc.gpsimd.tensor_scalar(hr[:, :tlen], srcs[c], 1.0, None, ALU.mult)
                    eng_q = nc.gpsimd if c == 2 else nc.vector
                    eng_q.tensor_mul(xq[:, :tlen], srcs[c], srcs[c])
                    hrs.append((hr, xq))
                ps_s = psp.tile([128, TT], F32, tag=pstag, name='ps_s')
                ps_q = psp.tile([128, TT], F32, tag=pstag, name='ps_q')
                for c in range(3):
                    nc.tensor.matmul(ps_s[:, :tlen], ones_t[:], hrs[c][0][:, :tlen],
                                     start=(c == 0), stop=(c == 2))
                for c in range(3):
                    nc.tensor.matmul(ps_q[:, :tlen], ones_t[:], hrs[c][1][:, :tlen],
                                     start=(c == 0), stop=(c == 2))
                if cgl is None:
                    A, Bc_ = 1.0, EPS
                else:
                    A = float(cgl) * float(cgl) + EPS
                    Bc_ = EPS * EPS
                import math
                sA = math.sqrt(A)
                st = stp.tile([128, 6, TT], F32, tag='st', name='st')
                m = st[:, 0, :tlen]
                mA = st[:, 1, :tlen]
                msqA = st[:, 2, :tlen]
                t1 = st[:, 3, :tlen]
                ve = st[:, 4, :tlen]
                u = st[:, 5, :tlen]
                rsd = st[:, 1, :tlen]   # mA dead after msqA
                nc.vector.tensor_scalar(m, ps_s[:, :tlen], 1.0 / D, None, ALU.mult)
                nc.vector.tensor_scalar(mA, ps_s[:, :tlen], sA / D, None, ALU.mult)
                nc.vector.tensor_mul(msqA, mA, mA)
                nc.vector.tensor_scalar(t1, ps_q[:, :tlen], A / D, Bc_, ALU.mult, ALU.add)
                nc.vector.tensor_sub(ve, t1, msqA)
                nc.vector.reciprocal(u, ve)
                nc.scalar.activation(rsd, u, AF.Sqrt)
                return st, m, rsd

            def ln_apply(st, m, rsd, srcs, dsts, tlen):
                tmp = st[:, 2, :tlen]
                tmp2 = st[:, 3, :tlen]
                for c in range(3):
                    eng = nc.gpsimd if c == 2 else nc.vector
                    tm = tmp2 if c == 2 else tmp
                    eng.tensor_sub(tm, srcs[c], m)
                    eng.tensor_mul(dsts[c], tm, rsd)

            # ================= front (streamed per 2-sample group) ==========
            with (
                tc.tile_pool(name='fgrp', bufs=2) as fg_,
                tc.tile_pool(name='fw', bufs=1) as fw,
            ):
                pft_t = fw.tile([6, NBP], F32R, name='pft_t')
                nc.sync.dma_start(pft_t[:], PFT[:])
                pew_t = fw.tile([128, 3, 2, 128], F32R, name='pew_t')
                nc.sync.dma_start(pew_t[:], PEWd[:])
                phw_t = fw.tile([6, 3, 128], F32R, name='phw_t')
                nc.sync.dma_start(phw_t[:], PHWd[:])
                for grp in range(4):
                    sl = slice(grp * BT, (grp + 1) * BT)
                    patg = fg_.tile([128, 3, 2, BT], F32R, tag='patg', name='patg')
                    for c in range(3):
                        nc.sync.dma_start(patg[:, c], PATd[:, c, :, sl])
                    ximg = fg_.tile([128, 3, BT], F32R, tag='ximg', name='ximg')
                    xn = fg_.tile([128, 3, BT], F32R, tag='xn', name='xn')
                    xp = fg_.tile([128, 3, BT], F32R, tag='xp', name='xp')
                    gt = fg_.tile([128, 3, BT], F32, tag='gt', name='gt')
                    for c in range(3):
                        ps_pe = psp.tile([128, TT], F32, tag='ps', name='ps_pe')
                        for kc in range(2):
                            nc.tensor.matmul(ps_pe[:, :BT], pew_t[:, c, kc, :], patg[:, c, kc, :],
                                             start=(kc == 0), stop=(kc == 1))
                        nc.scalar.activation(ximg[:, c, :], ps_pe[:, :BT], AF.Identity,
                                             bias=fbias_t[:, c:c + 1])
                    if grp == 0:
                        gw_t = fw.tile([128, 6, 3, 128], F32R, name='gw_t')
                        nc.sync.dma_start(gw_t[:], GWd[:])
                    xi = [ximg[:, c, :].bitcast(F32) for c in range(3)]
                    st, m, rsd = ln_stats(xi, BT)
                    ln_apply(st, m, rsd, xi, [xn[:, c, :] for c in range(3)], BT)
                    for mc in range(3):
                        ps_ph = psp.tile([128, TT], F32, tag='ps', name='ps_ph')
                        nc.tensor.matmul(ps_ph[:, :BT], phw_t[:, mc, :], pft_t[:, sl],
                                         start=True, stop=True)
                        nc.scalar.activation(xp[:, mc, :], ps_ph[:, :BT], AF.Identity,
                                             bias=fbias_t[:, 3 + mc:4 + mc])
                    for mc in range(3):
                        ps_g = psp.tile([128, TT], F32, tag='ps', name='ps_g')
                        for kc in range(6):
                            rhs = xn[:, kc, :] if kc < 3 else xp[:, kc - 3, :]
                            nc.tensor.matmul(ps_g[:, :BT], gw_t[:, kc, mc, :], rhs,
                                             start=(kc == 0), stop=(kc == 5))
                        nc.scalar.activation(gt[:, mc, :], ps_g[:, :BT], AF.Sigmoid,
                                             bias=fbias_t[:, 6 + mc:7 + mc])
                    for bl in range(2):
                        b = 2 * grp + bl
                        psl = slice(bl * NP_, (bl + 1) * NP_)
                        tsl = slice(b * S + 1, (b + 1) * S)
                        dd = stp.tile([128, 6, TT], F32, tag='st', name='fd')
                        dv = dd[:, 0:3, :NP_]
                        nc.vector.tensor_sub(dv, xn[:, :, psl].bitcast(F32), xp[:, :, psl].bitcast(F32))
                        nc.vector.tensor_mul(dv, gt[:, :, psl], dv)
                        nc.vector.tensor_add(dv, dv, xp[:, :, psl].bitcast(F32))
                        nc.vector.tensor_add(hT[:, :, tsl], dv, pet_t[:])
                        nc.vector.tensor_copy(hT[:, :, b * S:b * S + 1],
                                              fbias_t[:, 9:12].unsqueeze(2))

            # ========================= transformer layers ===================
            with (
                tc.tile_pool(name='wbig', bufs=3) as wb,
                tc.tile_pool(name='wps', bufs=1) as wps,
                tc.tile_pool(name='fgp', bufs=BUFS_FG) as fgp,
                tc.tile_pool(name='midp', bufs=BUFS_MID) as midp,
                tc.tile_pool(name='h2p', bufs=BUFS_H2) as h2p,
                tc.tile_pool(name='amp', bufs=1) as amp,
            ):
                pending = None
                for l in range(n_layers):
                    w1_t = wb.tile([128, 3, 1536], BF16, tag='w', name='w1_t')
                    nc.sync.dma_start(w1_t[:], W1d[l])
                    w2_t = wb.tile([128, 12, 3, 128], BF16, tag='w', name='w2_t')
                    nc.sync.dma_start(w2_t[:], W2d[l])
                    bd_t = wps.tile([128, 3, 4, 128], F32R, tag='bd', name='bd_t')
                    nc.sync.dma_start(bd_t[:], BDd[l])
                    aw1_t = wps.tile([128, 3, 384], F32, tag='aw1', name='aw1_t')
                    nc.sync.dma_start(aw1_t[:], AW1d[l])
                    aw2_t = wps.tile([128, 3, 1024], F32, tag='aw2', name='aw2_t')
                    nc.sync.dma_start(aw2_t[:], AW2d[l])
                    bias_t = wps.tile([128, 26], F32, tag='bias', name='bias_t')
                    nc.sync.dma_start(bias_t[:], BIASd[l])
                    ab2r_t = wps.tile([1, 1024], F32, tag='ab2r', name='ab2r_t')
                    nc.sync.dma_start(ab2r_t[:], AB2Rd[l])
                    b2r_t = wps.tile([1, 3, 128], BF16, tag='b2r', name='b2r_t')
                    nc.sync.dma_start(b2r_t[:], B2Rd[l])

                    hn = hnp.tile([128, 3, NTOK], F32R, tag='hn', name='hn')
                    mh = amp.tile([128, 3, BC], F32, tag='mh', name='mh')
                    if pending is None:
                        sts = []
                        for t in range(4):
                            sl = slice(t * TT, (t + 1) * TT)
                            hs = [hT[:, c, sl] for c in range(3)]
                            sts.append((sl, hs) + ln_stats(hs, TT, cgl=g['cg'][l]))
                    else:
                        sts = pending
                    ps_u = psp.tile([128, TT], F32, tag='ps', name='ps_u')
                    ps_e = psp.tile([128, TT], F32, tag='ps', name='ps_e')
                    u2t = amp.tile([128, 3, BC], F32, tag='u2', name='u2t')
                    eff = amp.tile([128, 8, BC], F32, tag='eff', name='eff')
                    for t in range(4):
                        sl, hs, st, m, rsd = sts[t]
                        ln_apply(st, m, rsd, hs, [hn[:, c, sl] for c in range(3)], TT)
                        bsl = slice(2 * t, 2 * t + 2)
                        for j in range(2):
                            b = 2 * t + j
                            for c in range(3):
                                eng_r = nc.vector
                                eng_r.reduce_sum(mh[:, c, b:b + 1],
                                                 hn[:, c, sl][:, j * S:(j + 1) * S].bitcast(F32),
                                                 axis=mybir.AxisListType.X)
                        for mc in range(3):
                            for kc in range(3):
                                nc.tensor.matmul(
                                    ps_u[:, mc * BC:mc * BC + BC][:, bsl],
                                    aw1_t[:, kc, mc * 128:(mc + 1) * 128],
                                    mh[:, kc, bsl], start=(kc == 0), stop=(kc == 2))
                        if t in (1, 3):
                            hsl = slice(0, 4) if t == 1 else slice(4, 8)
                            for mc in range(3):
                                nc.scalar.activation(u2t[:, mc, hsl],
                                                     ps_u[:, mc * BC:mc * BC + BC][:, hsl],
                                                     AF.Gelu, bias=bias_t[:, mc:mc + 1])
                            for mt in range(8):
                                for kc in range(3):
                                    nc.tensor.matmul(
                                        ps_e[:, mt * BC:mt * BC + BC][:, hsl],
                                        aw2_t[:, kc, mt * 128:(mt + 1) * 128],
                                        u2t[:, kc, hsl], start=(kc == 0), stop=False)
                                nc.tensor.matmul(
                                    ps_e[:, mt * BC:mt * BC + BC][:, hsl],
                                    ab2r_t[:, mt * 128:(mt + 1) * 128],
                                    onesf_t[0:1, hsl], start=False, stop=True)
                                nc.vector.tensor_scalar(eff[:, mt, hsl],
                                                        ps_e[:, mt * BC:mt * BC + BC][:, hsl],
                                                        1.0, None, ALU.mult)


                    # FFT mixer
                    KCS_F = [[0], [0, 1], [1, 2], [2]]
                    KCS_I = [[0, 1], [1, 2], [2, 3]]
                    for t in range(4):
                        sl = slice(t * TT, (t + 1) * TT)
                        fg = fgp.tile([128, 4, TT], F32R, tag='fg', name='fg')
                        for mc in range(4):
                            ps_F = psp.tile([128, TT], F32, tag='ps', name='ps_F')
                            kcs = KCS_F[mc]
                            for i, kc in enumerate(kcs):
                                nc.tensor.matmul(ps_F[:], bd_t[:, kc, mc, :], hn[:, kc, sl],
                                                 start=(i == 0), stop=(i == len(kcs) - 1))
                            for j in range(2):
                                bb = 2 * t + j
                                nc.scalar.activation(fg[:, mc, j * S:(j + 1) * S],
                                                     ps_F[:, j * S:(j + 1) * S], AF.Gelu,
                                                     scale=eff[:, mc, bb:bb + 1],
                                                     bias=eff[:, 4 + mc, bb:bb + 1])
                        for mc in range(3):
                            ps_A = psp.tile([128, TT], F32, tag='ps', name='ps_A')
                            kcs = KCS_I[mc]
                            for i, kc in enumerate(kcs):
                                nc.tensor.matmul(ps_A[:], ibd_t[:, kc, mc, :], fg[:, kc, :],
                                                 start=(i == 0), stop=(i == len(kcs) - 1))
                            nc.vector.tensor_add(hT[:, mc, sl], hT[:, mc, sl], ps_A[:])

                    # LN2 + MLP (stats pipelined one tile ahead); LN1 stats of
                    # the NEXT layer are emitted per tile right after its
                    # residual lands, overlapping this layer's MLP matmuls.
                    ln2q = []
                    for t in range(4):
                        sl = slice(t * TT, (t + 1) * TT)
                        hs = [hT[:, c, sl] for c in range(3)]
                        ln2q.append((sl, hs) + ln_stats(hs, TT))
                    nxt = []
                    for t in range(4):
                        sl, hs, st, m, rsd = ln2q[t]
                        h2 = h2p.tile([128, 3, TT], BF16, tag='h2', name='h2')
                        ln_apply(st, m, rsd, hs, [h2[:, c, :] for c in range(3)], TT)

                        mid = midp.tile([128, 12, TT], BF16, tag='mid', name='mid')
                        for grp in range(3):
                            pss = []
                            for mci in range(4):
                                mc = grp * 4 + mci
                                ps_m = psp.tile([128, TT], F32, tag='ps', name='ps_m')
                                for kc in range(3):
                                    nc.tensor.matmul(ps_m[:], w1_t[:, kc, mc * 128:(mc + 1) * 128],
                                                     h2[:, kc, :], start=(kc == 0), stop=(kc == 2))
                                pss.append((mc, ps_m))
                            for mc, ps_m in pss:
                                nc.scalar.activation(mid[:, mc, :], ps_m[:], AF.Gelu,
                                                     bias=bias_t[:, 11 + mc:12 + mc])
                        for mc in range(3):
                            ps_o = psp.tile([128, TT], F32, tag='ps', name='ps_o')
                            for kc in range(12):
                                nc.tensor.matmul(ps_o[:], w2_t[:, kc, mc, :], mid[:, kc, :],
                                                 start=(kc == 0), stop=False)
                            nc.tensor.matmul(ps_o[:], b2r_t[:, mc, :], onesb_t[0:1, :TT],
                                             start=False, stop=True)
                            nc.vector.tensor_add(hT[:, mc, sl], hT[:, mc, sl], ps_o[:])
                        if l + 1 < n_layers:
                            nxt.append((sl, hs) + ln_stats(hs, TT, cgl=g['cg'][l + 1]))
                    pending = nxt if l + 1 < n_layers else None

                out_ap = hT[:].rearrange("p c (b s) -> p c b s", s=S)[:, :, :, 0]
                nc.sync.dma_start(HCLS[:], out_ap)

    nc.compile()
    return nc


def _gelu_np(x):
    try:
        from scipy.special import erf
    except ImportError:
        import math
        erf = np.vectorize(math.erf)
    return x * 0.5 * (1.0 + erf(x / np.sqrt(2.0)))


def _head(hcls, g):
    x = hcls.astype(np.float64).T
    m = x.mean(1, keepdims=True)
    v = ((x - m) ** 2).mean(1, keepdims=True)
    cls = (x - m) / np.sqrt(v + EPS) * g['norm_g'] + g['norm_b']
    u = _gelu_np(cls @ g['head_w1'] + g['head_b1'])
    return ((u @ g['head_w2'])[:, 0] + g['head_b2'][0]).astype(np.float32)


def _in_maps(inputs, g):
    x = np.ascontiguousarray(inputs['x'], np.float32)
    pf = np.ascontiguousarray(inputs['patch_feats'], np.float32)
    shared = dict(
        w1=g['W1'], w2=g['W2'], bd=g['BD'], ibd=g['IBD'], aw1=g['AW1'],
        aw2=g['AW2'], bias=g['BIAS'], ab2r=g['AB2R'], b2r=g['B2R'],
        onesf=np.ones((1, BC), np.float32),
        onesb=_bf16(np.ones((1, TT))), pew=g['PEW'], phw=g['PHW'], gw=g['GW'],
        fbias=g['FBIAS'], pet=g['PET'],
        ones=np.ones((128, 128), np.float32),
    )
    Hp = 224 // P
    pat = x.reshape(B, 3, Hp, P, Hp, P).transpose(0, 1, 2, 4, 3, 5).reshape(B, 3, NP_, 2, 128)
    maps = []
    for i in range(NCORES):
        m = dict(shared)
        pc = pat[i * BC:(i + 1) * BC]                       # [BC,3,196,2,128]
        m['patt'] = np.ascontiguousarray(pc.transpose(4, 1, 3, 0, 2).reshape(128, 3, 2, NBP))
        m['pft'] = np.ascontiguousarray(pf[i * BC:(i + 1) * BC].reshape(NBP, 6).T)
        maps.append(m)
    return maps


def kernel(**inputs):
    inputs = {k: np.asarray(v) for k, v in inputs.items()}
    g = _prep(inputs)
    # program structure bakes per-layer ln1 gains into immediates; key on them
    key = (tuple(np.round(np.asarray(g['cg'], np.float64), 12)),)
    if _CACHE.get('key') != key:
        _CACHE['prog'] = _build(g)
        _CACHE['key'] = key
    nc = _CACHE['prog']
    res = run_bass_kernel_spmd(nc, _in_maps(inputs, g), list(range(NCORES)))
    _CACHE['last_res'] = res
    _CACHE['last_g'] = g
    hcls = np.concatenate(
        [r['hcls'].transpose(1, 0, 2).reshape(D, BC) for r in res.results], axis=1)
    return _head(hcls, g)


if __name__ == '__main__':
    d = np.load('/root/problem/ref_data.npz')
    inputs = {k: d[k] for k in d.files if k != 'expected'}
    y = kernel(**inputs)
    exp = d['expected']
    err = np.abs(y - exp)
    print("max abs err:", err.max())
    print("Relative error:", err.max() / np.abs(exp).max())
